# revision 1
# baseline (speedup 1.0000x reference)
"""ClusterAttention Trainium2 kernel (v3): three device phases.

A0  (token order, core = half of one batch element): qkv projection
    [384]->[1152] for 4096 tokens, bf16 matmuls, psum drained round-robin
    across DVE/Pool/Act into a staging tile, 9 batched DMAs out.
host: gather q/k/v into per-(b,h)-row cluster order (permutation only),
    prefill aux rows (q: ones, k: s_j + b_pos) and the v ones-column,
    pre-transpose v into token-major AV layout. The -s_i pos-bias term is
    constant along the softmax axis, so it is dropped entirely.
A1  (cluster order, core = 6 rows): S[j,i] = k_aug.T @ q_aug (17-dim
    contraction, 4 clusters per 2-bank psum tile), exp on Act -> E bf16,
    AV with ones-column denominator, unnormalized bf16 out. AV lags one
    group behind S/exp so the in-order PE queue never waits on Act.
host: normalize by the denominator column + scatter to token order.
B   (token order, core = half batch x half tokens): [768]->[384] out
    projection, bf16, one batched DMA per 512-token tile each way.
    Bias is added on the host.
"""
import numpy as np
import ml_dtypes

import concourse.bacc as bacc
import concourse.tile as tile
from concourse import mybir
from concourse.bass_utils import run_bass_kernel_spmd

B, N, C, H, D, K, M = 4, 8192, 384, 12, 2, 32, 256
CH = C // H // 2            # 16
BH = B * H                  # 48
R = BH // 8                 # 6 rows per core in A1
SCALE = float((C // H) ** -0.5)
TPA = N // 2                # 4096 tokens per core in A0
QKV = 3 * C                 # 1152 projected dims
TPB = N * B // 8            # 4096 tokens per core in phase B
G = K // 4                  # 8 cluster groups of 4 per row in A1

F32 = mybir.dt.float32
BF16 = mybir.dt.bfloat16
NPBF = ml_dtypes.bfloat16


def build_phase_a0():
    nc = bacc.Bacc(None, target_bir_lowering=False)
    ft_d = nc.dram_tensor("ft", [3 * 128, TPA], BF16, kind="ExternalInput")
    w_d = nc.dram_tensor("w", [3 * 128, QKV], BF16, kind="ExternalInput")
    qkvT = nc.dram_tensor("qkvT", [9 * 128, TPA], BF16, kind="ExternalOutput")

    with tile.TileContext(nc) as tc:
        with (
            tc.tile_pool(name="sb", bufs=1) as pool,
            tc.tile_pool(name="ps", bufs=4, space="PSUM") as ps,
        ):
            ft = pool.tile([128, 3 * TPA], BF16, tag="ft")
            w_sb = pool.tile([128, 3 * QKV], BF16, tag="w_sb")
            qv = qkvT.rearrange("(c p) n -> p c n", p=128)
            # token-major compute (oc inner) consumes 512 cols per ~5.8us while
            # the DMA delivers them in ~2us -> the PE never races the loads
            for cc in range(3):
                nc.sync.dma_start(ft[:, cc * TPA: cc * TPA + 512],
                                  ft_d[cc * 128:(cc + 1) * 128, 0:512])
            for cc in range(3):
                nc.sync.dma_start(w_sb[:, cc * QKV:(cc + 1) * QKV],
                                  w_d[cc * 128:(cc + 1) * 128, :])
            for cc in range(3):
                nc.sync.dma_start(ft[:, cc * TPA + 512: cc * TPA + TPA],
                                  ft_d[cc * 128:(cc + 1) * 128, 512:])
            NTT = TPA // 512
            for tt in range(NTT):
                t0 = tt * 512
                stg = pool.tile([128, 9 * 512], BF16, tag="stg", bufs=3)
                sv = stg.rearrange("p (c w) -> p c w", w=512)
                for oc in range(9):
                    p = ps.tile([128, 512], F32, tag="p")
                    for cc in range(3):
                        nc.tensor.matmul(
                            p[:, :],
                            w_sb[:, cc * QKV + oc * 128: cc * QKV + (oc + 1) * 128],
                            ft[:, cc * TPA + t0: cc * TPA + t0 + 512],
                            start=(cc == 0), stop=(cc == 2))
                    dst = stg[:, oc * 512:(oc + 1) * 512]
                    if oc % 2 == 0:
                        nc.vector.tensor_copy(dst, p[:, :])
                    else:
                        nc.scalar.activation(dst, p[:, :],
                                             mybir.ActivationFunctionType.Copy)
                    # last tile: drain in 3-chunk slices to shorten the tail
                    if tt == NTT - 1 and oc % 3 == 2:
                        nc.sync.dma_start(
                            qv[:, oc - 2: oc + 1, t0:t0 + 512],
                            sv[:, oc - 2: oc + 1, :])
                if tt < NTT - 1:
                    nc.sync.dma_start(qv[:, :, t0:t0 + 512], sv)
    nc.compile()
    return nc


def build_phase_a1():
    nc = bacc.Bacc(None, target_bir_lowering=False)
    qk_g = nc.dram_tensor("qk_g", [R * 34, N], BF16, kind="ExternalInput")
    v_g = nc.dram_tensor("v_g", [R * 128, 64 * 65], BF16, kind="ExternalInput")
    out_g = nc.dram_tensor("out_g", [R * N, 65], BF16, kind="ExternalOutput")

    with tile.TileContext(nc) as tc:
        with (
            tc.tile_pool(name="sb_qk", bufs=2) as p_qk,
            tc.tile_pool(name="sb_v", bufs=2) as p_v,
            tc.tile_pool(name="sb_e", bufs=8) as p_e,
            tc.tile_pool(name="sb_o", bufs=4) as p_o,
            tc.tile_pool(name="ps_s", bufs=2, space="PSUM") as ps_sp,
            tc.tile_pool(name="ps_av", bufs=4, space="PSUM") as ps_av,
        ):
            tiles = {}

            def alloc_row(r):
                q_sb = p_qk.tile([17, N], BF16, tag="q_sb")
                k_sb = p_qk.tile([17, N], BF16, tag="k_sb")
                v_sb = p_v.tile([128, 64 * 65], BF16, tag="v_sb")
                hn, hv = N // 2, 64 * 65 // 2
                for h4 in range(2):
                    nc.sync.dma_start(q_sb[:, h4 * hn:(h4 + 1) * hn],
                                      qk_g[r * 34: r * 34 + 17, h4 * hn:(h4 + 1) * hn])
                    nc.sync.dma_start(k_sb[:, h4 * hn:(h4 + 1) * hn],
                                      qk_g[r * 34 + 17: r * 34 + 34,
                                           h4 * hn:(h4 + 1) * hn])
                    nc.sync.dma_start(v_sb[:, h4 * hv:(h4 + 1) * hv],
                                      v_g[r * 128:(r + 1) * 128,
                                          h4 * hv:(h4 + 1) * hv])
                tiles[r] = (q_sb, k_sb, v_sb.rearrange("p (c w) -> p c w", w=65))

            def s_exp(r, g):
                q_sb, k_sb, _ = tiles[r]
                e_tiles = []
                for jc in range(2):
                    ps_s = ps_sp.tile([128, 1024], F32, tag="ps_s")
                    for u in range(4):
                        col = (g * 4 + u) * 256
                        nc.tensor.matmul(
                            ps_s[:, u * 256:(u + 1) * 256],
                            k_sb[0:17, col + jc * 128: col + (jc + 1) * 128],
                            q_sb[0:17, col: col + 256],
                            start=True, stop=True)
                    et = p_e.tile([128, 1024], BF16, tag="e")
                    nc.scalar.activation(et[:, :], ps_s[:, :],
                                         mybir.ActivationFunctionType.Exp)
                    e_tiles.append(et)
                return e_tiles

            def av_out(r, g, e_tiles):
                _, _, v_view = tiles[r]
                o_sb = p_o.tile([128, 2 * 260], BF16, tag="o_sb")
                for pair in range(2):
                    ps_o = ps_av.tile([128, 260], F32, tag="ps_o")
                    for u in range(2):
                        kk = g * 4 + pair * 2 + u
                        for ic in range(2):
                            sl = (u * 2 + ic) * 65
                            ecol = (pair * 2 + u) * 256 + ic * 128
                            for jc in range(2):
                                nc.tensor.matmul(
                                    ps_o[:, sl:sl + 65],
                                    e_tiles[jc][:, ecol: ecol + 128],
                                    v_view[:, kk * 2 + jc, :],
                                    start=(jc == 0), stop=(jc == 1))
                    nc.vector.tensor_copy(o_sb[:, pair * 260:(pair + 1) * 260],
                                          ps_o[:, :])
                row0 = r * N + g * 1024
                nc.sync.dma_start(
                    out_g[row0: row0 + 1024, :].rearrange("(c p) w -> p c w", p=128),
                    o_sb.rearrange("p (c w) -> p c w", w=65))

            # global AV lag of TWO cluster groups behind S/exp (also across
            # row boundaries): the in-order PE always has met dependencies,
            # so the Act engine (the bottleneck) is never starved
            from collections import deque
            alloc_row(0)
            pend = deque()
            for r in range(R):
                for g in range(G):
                    if len(pend) >= 2:
                        done = pend.popleft()
                        av_out(*done)
                        if done[1] == G - 1:
                            del tiles[done[0]]
                    pend.append((r, g, s_exp(r, g)))
                    if g == 2 and r + 1 < R:
                        # row r-1's tiles retired (slot reused by row r+1)
                        alloc_row(r + 1)
            while pend:
                done = pend.popleft()
                av_out(*done)
    nc.compile()
    return nc


def build_phase_b():
    nc = bacc.Bacc(None, target_bir_lowering=False)
    f2T = nc.dram_tensor("f2T", [6 * 128, TPB], BF16, kind="ExternalInput")
    wp2 = nc.dram_tensor("wp2", [6 * 128, 384], BF16, kind="ExternalInput")
    outT = nc.dram_tensor("outT", [3 * 128, TPB], BF16, kind="ExternalOutput")
    f2v = f2T.rearrange("(c p) n -> p c n", p=128)
    ov = outT.rearrange("(c p) n -> p c n", p=128)

    with tile.TileContext(nc) as tc:
        with (
            tc.tile_pool(name="sb", bufs=1) as pool,
            tc.tile_pool(name="sb_f", bufs=3) as p_f,
            tc.tile_pool(name="sb_o", bufs=4) as p_o,
            tc.tile_pool(name="ps", bufs=4, space="PSUM") as ps,
        ):
            wsb = pool.tile([128, 6 * 384], BF16, tag="wsb")
            wv_ = wsb.rearrange("p (c w) -> p c w", w=384)
            wp_ = wp2.rearrange("(c p) n -> p c n", p=128)
            NTB = TPB // 512
            fsb0 = p_f.tile([128, 6 * 512], BF16, tag="fsb")
            nc.sync.dma_start(wv_[:, :, 0:128], wp_[:, :, 0:128])
            nc.sync.dma_start(fsb0.rearrange("p (c w) -> p c w", w=512),
                              f2v[:, :, 0:512])
            nc.sync.dma_start(wv_[:, :, 128:384], wp_[:, :, 128:384])
            for tt in range(NTB):
                t0 = tt * 512
                if tt == 0:
                    fsb = fsb0
                else:
                    fsb = p_f.tile([128, 6 * 512], BF16, tag="fsb")
                    nc.sync.dma_start(
                        fsb.rearrange("p (c w) -> p c w", w=512),
                        f2v[:, :, t0:t0 + 512])
                osb = p_o.tile([128, 3 * 512], BF16, tag="osb")
                ov_s = osb.rearrange("p (c w) -> p c w", w=512)
                for oc in range(3):
                    p = ps.tile([128, 512], F32, tag="p")
                    for cc in range(6):
                        nc.tensor.matmul(
                            p[:, :],
                            wsb[:, cc * 384 + oc * 128: cc * 384 + (oc + 1) * 128],
                            fsb[:, cc * 512: (cc + 1) * 512],
                            start=(cc == 0), stop=(cc == 5))
                    if oc == 2:
                        nc.vector.tensor_copy(osb[:, oc * 512:(oc + 1) * 512],
                                              p[:, :])
                    else:
                        nc.scalar.activation(osb[:, oc * 512:(oc + 1) * 512],
                                             p[:, :],
                                             mybir.ActivationFunctionType.Copy)
                    # last tile: drain per-oc to shorten the tail
                    if tt == NTB - 1:
                        nc.sync.dma_start(ov[:, oc:oc + 1, t0:t0 + 512],
                                          ov_s[:, oc:oc + 1, :])
                if tt < NTB - 1:
                    nc.sync.dma_start(ov[:, :, t0:t0 + 512], ov_s)
    nc.compile()
    return nc


_CACHE = {}


def _get(name, builder):
    if name not in _CACHE:
        _CACHE[name] = builder()
    return _CACHE[name]


def kernel(pos, feat, member_idx, w_qkv, b_qkv, w_pos, b_pos, w_proj, b_proj):
    import os, time
    pos = np.asarray(pos, np.float32)
    feat = np.asarray(feat, np.float32)
    mf = np.asarray(member_idx).astype(np.int64).reshape(BH, N)
    w_qkv = np.asarray(w_qkv, np.float32); b_qkv = np.asarray(b_qkv, np.float32)
    w_pos = np.asarray(w_pos, np.float32); b_pos = np.asarray(b_pos, np.float32)
    w_proj = np.asarray(w_proj, np.float32); b_proj = np.asarray(b_proj, np.float32)

    t0_ = time.time()
    # ---- A0 prep: token-order feat (c-major) + fused weight, bf16
    featT = np.ascontiguousarray(feat.transpose(0, 2, 1)).astype(NPBF)   # [B,C,N]
    # W columns: [q all heads (scaled) | k all heads | v all heads]
    Wbig = np.empty((C, QKV), np.float32)
    for h in range(H):
        Wbig[:, h * 16:(h + 1) * 16] = SCALE * w_qkv[h * 96: h * 96 + 16].T
        Wbig[:, 192 + h * 16: 192 + (h + 1) * 16] = w_qkv[h * 96 + 16: h * 96 + 32].T
        Wbig[:, 384 + h * 64: 384 + (h + 1) * 64] = w_qkv[h * 96 + 32: h * 96 + 96].T
    Wb = Wbig.astype(NPBF)
    in_maps_a0 = []
    for c in range(8):
        b, half = divmod(c, 2)
        in_maps_a0.append({
            "ft": np.ascontiguousarray(featT[b][:, half * TPA:(half + 1) * TPA]),
            "w": Wb,
        })
    nc_a0 = _get("a0", build_phase_a0)
    t_run0 = time.time()
    res_a0 = run_bass_kernel_spmd(nc_a0, in_maps_a0, core_ids=list(range(8)))
    t_run1 = time.time()

    qkv_b = [np.concatenate([res_a0.results[2 * b]["qkvT"],
                             res_a0.results[2 * b + 1]["qkvT"]], axis=1)
             for b in range(B)]                                          # [1152,N] bf16

    # ---- host gather into cluster order per (b,h) row
    pos_n = pos / pos.reshape(-1, D).max(0)
    b_of = np.repeat(np.arange(B), H)
    pos_g = np.take_along_axis(pos_n[b_of], mf[:, :, None], axis=1)      # [48,N,2]
    s_all = np.einsum('rnd,rd->rn', pos_g, np.tile(w_pos, (B, 1)))       # [48,N]

    qk_all = np.empty((BH, 34, N), NPBF)
    v_all = np.empty((BH, 128, 64 * 65), NPBF)
    vtmp = np.ones((BH, 128, 64, 65), np.float32)
    for r in range(BH):
        b, h = divmod(r, H)
        qg = np.take(qkv_b[b][h * 16:(h + 1) * 16], mf[r], axis=1)
        kg = np.take(qkv_b[b][192 + h * 16: 192 + (h + 1) * 16], mf[r], axis=1)
        vg = np.take(qkv_b[b][384 + h * 64: 384 + (h + 1) * 64], mf[r], axis=1)
        qk_all[r, 0:16] = qg
        qk_all[r, 16] = 1.0
        qk_all[r, 17:33] = kg
        aux = s_all[r] + b_pos[h]
        bq = b_qkv[h * 96: h * 96 + 16]
        bk = b_qkv[h * 96 + 16: h * 96 + 32]
        if np.any(bq) or np.any(bk):
            # logit = scale*(q+bq).(k+bk) + s_j + b_pos; the i-only term
            # scale*(q_i.bk) is constant along the softmax axis -> dropped.
            aux = aux + SCALE * (bq @ kg.astype(np.float32)) + SCALE * float(bq @ bk)
        qk_all[r, 33] = aux
        # v -> token-major AV layout [p, chunk, c], ones at c=64
        vtmp[r, :, :, 0:64] = vg.reshape(64, 64, 128).transpose(2, 1, 0)
    v_all[:] = vtmp.reshape(BH, 128, 64 * 65)

    in_maps_a1 = []
    for c in range(8):
        rs = slice(c * R, (c + 1) * R)
        in_maps_a1.append({
            "qk_g": qk_all[rs].reshape(R * 34, N),
            "v_g": v_all[rs].reshape(R * 128, 64 * 65),
        })
    nc_a1 = _get("a1", build_phase_a1)
    t_run2 = time.time()
    res_a1 = run_bass_kernel_spmd(nc_a1, in_maps_a1, core_ids=list(range(8)))
    t_run3 = time.time()

    out_g_all = np.concatenate(
        [res_a1.results[c]["out_g"].reshape(R, N, 65) for c in range(8)],
        axis=0).astype(np.float32)

    # ---- host: softmax normalize + scatter to token order, build feat2T
    out_n = out_g_all[:, :, 0:64] / out_g_all[:, :, 64:65]               # [48,N,64]
    f2T = np.empty((B, 2 * C, N), NPBF)
    for r in range(BH):
        b, h = divmod(r, H)
        f2T[b, h * 64:(h + 1) * 64, mf[r]] = out_n[r]
    wp2 = np.ascontiguousarray(w_proj.T).astype(NPBF)                    # [768,384]
    b_eff = b_proj + w_proj[:, :] @ np.concatenate(
        [b_qkv[h * 96 + 32: h * 96 + 96] for h in range(H)])
    in_maps_b = []
    for c in range(8):
        b, half = divmod(c, 2)
        tsl = slice(half * TPB, (half + 1) * TPB)
        in_maps_b.append({"f2T": np.ascontiguousarray(f2T[b][:, tsl]), "wp2": wp2})
    nc_b = _get("b", build_phase_b)
    t_run4 = time.time()
    res_b = run_bass_kernel_spmd(nc_b, in_maps_b, core_ids=list(range(8)))
    t_run5 = time.time()

    out = np.empty((B, N, C), np.float32)
    for c in range(8):
        b, half = divmod(c, 2)
        out[b, half * TPB:(half + 1) * TPB, :] = \
            res_b.results[c]["outT"].astype(np.float32).T + b_eff[None, :]
    if os.environ.get("KTIME"):
        print(f"[kernel] prep={t_run0-t0_:.2f}s runA0={t_run1-t_run0:.2f}s "
              f"gather={t_run2-t_run1:.2f}s runA1={t_run3-t_run2:.2f}s "
              f"scatter={t_run4-t_run3:.2f}s runB={t_run5-t_run4:.2f}s")
    return out



# revision 32
# speedup vs baseline: 1.1430x; 1.1430x over previous
"""ClusterAttention Trainium2 kernel (v4): three device phases.

A0  (token order, core = half of one batch element): qkv projection
    [384]->[1152] for 4096 tokens, bf16 matmuls. PE warm-up matmuls run
    on a zeroed tile during the initial DMA wait so real matmuls start
    at full clock. Weights arrive via the Act DGE queue in one DMA while
    the first feat chunk loads on the SP queue. PSUM drains round-robin
    across DVE/Act/Pool; output DMAs are batched per 3-oc slice.
host: gather q/k/v into per-(b,h)-row cluster order (permutation only),
    prefill aux rows (q: ones, k: s_j + b_pos) and the v ones-column,
    pre-transpose v into token-major AV layout. The -s_i pos-bias term is
    constant along the softmax axis, so it is dropped entirely.
A1  (cluster order, core = 6 rows): S[j,i] = k_aug.T @ q_aug (17-dim
    contraction, 4 clusters per 2-bank psum tile). exp runs on Act for
    5 of 8 cluster groups and on DVE for the other 3 via a one-op
    Schraudolph fast-exp: int16(S*128/ln2 + (127-C)*128) reinterpreted
    as bf16 IS exp(S) to ~3%; softmax renormalization cancels most of
    the common-mode error (measured end-to-end rel err ~1.4e-2 vs the
    2e-2 gate). AV consumes either E tile through a bitcast view. AV
    lags two groups so the in-order PE queue never stalls on exp. AV
    psum drains go to Pool (2/3) and DVE (1/3); the row output is
    staged [128, 64*65] and leaves in ONE v-layout-mirrored DMA per row
    (512B+ contiguous elements - no small-element DMA penalty), issued
    from the Pool queue to keep the SP queue short.
host: normalize by the denominator column + scatter to token order.
B   (token order, core = half batch x half tokens): [768]->[384] out
    projection, bf16, warm-up matmuls, weights on the Act queue, drains
    round-robin, one batched DMA per 512-token tile each way.
    Bias is added on the host.
"""
import numpy as np
import ml_dtypes

import concourse.bacc as bacc
import concourse.tile as tile
from concourse import mybir
from concourse.bass_utils import run_bass_kernel_spmd
from concourse.alu_op_type import AluOpType

B, N, C, H, D, K, M = 4, 8192, 384, 12, 2, 32, 256
CH = C // H // 2            # 16
BH = B * H                  # 48
R = BH // 8                 # 6 rows per core in A1
SCALE = float((C // H) ** -0.5)
TPA = N // 2                # 4096 tokens per core in A0
QKV = 3 * C                 # 1152 projected dims
TPB = N * B // 8            # 4096 tokens per core in phase B
G = K // 4                  # 8 cluster groups of 4 per row in A1

F32 = mybir.dt.float32
BF16 = mybir.dt.bfloat16
I16 = mybir.dt.int16
NPBF = ml_dtypes.bfloat16

# Schraudolph fast-exp constants for the bf16/int16 variant (round-to-
# nearest convert, calibrated offline: max rel err 3.27% on [-9, 9])
EXP_A = float(128.0 / np.log(2))
EXP_B = float(127.0 * 128 - 0.044 * 128)
# cluster groups whose exp runs on DVE (per row); rest on Act
import os as _os
DVE_GROUPS = tuple(int(x) for x in _os.environ.get("K_DVEG", "1,4,7").split(",") if x != "")
K_LAG = int(_os.environ.get("K_LAG", "2"))
K_EBUFS = int(_os.environ.get("K_EBUFS", "16"))
K_EIBUFS = int(_os.environ.get("K_EIBUFS", "12"))
K_PSS = int(_os.environ.get("K_PSS", "4"))
K_PSAV2 = int(_os.environ.get("K_PSAV2", "4"))
K_DRAIN = _os.environ.get("K_DRAIN", "pool")  # pool|mix


def _warmup(nc, pool, ps_pool, n_mm=8, cols=512, ps_tag="warm_ps", ps_cols=None):
    """Back-to-back matmuls on a zeroed tile: holds the PE busy through
    the p-state ramp while the first input DMAs land."""
    wz = pool.tile([128, cols], BF16, tag="warm")
    nc.vector.memset(wz[:, :], 0)
    pw = ps_pool.tile([128, ps_cols or cols], F32, tag=ps_tag)
    for _ in range(n_mm):
        nc.tensor.matmul(pw[:, 0:cols], wz[:, 0:128], wz[:, :], start=True, stop=True)


def build_phase_a0():
    nc = bacc.Bacc(None, target_bir_lowering=False)
    ft_d = nc.dram_tensor("ft", [3 * 128, TPA], BF16, kind="ExternalInput")
    w_d = nc.dram_tensor("w", [3 * 128, QKV], BF16, kind="ExternalInput")
    qkvT = nc.dram_tensor("qkvT", [9 * 128, TPA], BF16, kind="ExternalOutput")

    with tile.TileContext(nc) as tc:
        with (
            tc.tile_pool(name="sb", bufs=1) as pool,
            tc.tile_pool(name="sb_s", bufs=3) as p_s,
            tc.tile_pool(name="ps", bufs=4, space="PSUM") as ps,
            tc.tile_pool(name="ps_w", bufs=1, space="PSUM") as ps_w,
        ):
            ft = pool.tile([128, 3 * TPA], BF16, tag="ft")
            w_sb = pool.tile([128, 3 * QKV], BF16, tag="w_sb")
            qv = qkvT.rearrange("(c p) n -> p c n", p=128)
            wv = w_sb.rearrange("p (c w) -> p c w", w=QKV)
            _warmup(nc, pool, ps_w)
            # weights on the Act DGE queue, first feat chunk on SP: they
            # overlap; compute starts as soon as both land
            nc.scalar.dma_start(wv[:, :, :],
                                w_d.rearrange("(c p) n -> p c n", p=128))
            for cc in range(3):
                nc.sync.dma_start(ft[:, cc * TPA: cc * TPA + 512],
                                  ft_d[cc * 128:(cc + 1) * 128, 0:512])
            for cc in range(3):
                nc.sync.dma_start(ft[:, cc * TPA + 512: cc * TPA + TPA],
                                  ft_d[cc * 128:(cc + 1) * 128, 512:])
            NTT = TPA // 512
            dr = 0
            for tt in range(NTT):
                t0 = tt * 512
                stg = p_s.tile([128, 9 * 512], BF16, tag="stg")
                sv = stg.rearrange("p (c w) -> p c w", w=512)
                for oc in range(9):
                    p = ps.tile([128, 512], F32, tag="p")
                    for cc in range(3):
                        nc.tensor.matmul(
                            p[:, :],
                            w_sb[:, cc * QKV + oc * 128: cc * QKV + (oc + 1) * 128],
                            ft[:, cc * TPA + t0: cc * TPA + t0 + 512],
                            start=(cc == 0), stop=(cc == 2))
                    dst = stg[:, oc * 512:(oc + 1) * 512]
                    last = (tt == NTT - 1 and oc == 8)
                    if last:
                        # oc6/7 leave first; split oc8's drain+DMA in half
                        # so the final DMA is small
                        nc.sync.dma_start(qv[:, 6:8, t0:t0 + 512], sv[:, 6:8, :])
                        nc.vector.tensor_copy(stg[:, oc * 512: oc * 512 + 256],
                                              p[:, 0:256])
                        nc.sync.dma_start(qv[:, oc:oc + 1, t0:t0 + 256],
                                          sv[:, oc:oc + 1, 0:256])
                        nc.scalar.activation(stg[:, oc * 512 + 256:(oc + 1) * 512],
                                             p[:, 256:512],
                                             mybir.ActivationFunctionType.Copy)
                        nc.sync.dma_start(qv[:, oc:oc + 1, t0 + 256:t0 + 512],
                                          sv[:, oc:oc + 1, 256:512])
                    elif dr % 2 == 0:
                        nc.vector.tensor_copy(dst, p[:, :])
                    else:
                        nc.scalar.activation(dst, p[:, :],
                                             mybir.ActivationFunctionType.Copy)
                    dr += 1
                    # drain in 3-oc slices to keep the tail short
                    if not last and oc % 3 == 2 and not (tt == NTT - 1 and oc == 8):
                        nc.sync.dma_start(
                            qv[:, oc - 2: oc + 1, t0:t0 + 512],
                            sv[:, oc - 2: oc + 1, :])
    nc.compile()
    return nc


def build_phase_a1():
    nc = bacc.Bacc(None, target_bir_lowering=False)
    qk_g = nc.dram_tensor("qk_g", [R * 34, N], BF16, kind="ExternalInput")
    v_g = nc.dram_tensor("v_g", [R * 128, 64 * 65], BF16, kind="ExternalInput")
    # output mirrors the v layout: [row*128 + p, chunk(=2*cluster+ihalf)*65 + c]
    out_g = nc.dram_tensor("out_g", [R * 128, 64 * 65], BF16, kind="ExternalOutput")

    with tile.TileContext(nc) as tc:
        with (
            tc.tile_pool(name="sb_qk", bufs=2) as p_qk,
            tc.tile_pool(name="sb_v", bufs=2) as p_v,
            tc.tile_pool(name="sb_e", bufs=K_EBUFS) as p_e,
            tc.tile_pool(name="sb_ei", bufs=K_EIBUFS) as p_ei,
            tc.tile_pool(name="sb_o", bufs=2) as p_o,
            tc.tile_pool(name="sb_w", bufs=1) as p_w,
            tc.tile_pool(name="ps_s", bufs=K_PSS, space="PSUM") as ps_sp,
            tc.tile_pool(name="ps_av", bufs=K_PSAV2, space="PSUM") as ps_av,
        ):
            tiles = {}
            ostage = {}
            # warm psum borrows an AV-pool slot; first real ps_o reuse just
            # serializes behind the warm matmuls (done during the DMA wait)
            _warmup(nc, p_w, ps_av, n_mm=12, cols=260, ps_tag="ps_o")
            # preload the exp table set while input DMAs are in flight
            wpre = p_w.tile([128, 8], BF16, tag="wpre")
            nc.vector.memset(wpre[:, :], 0)
            nc.scalar.activation(wpre[:, 0:4], wpre[:, 4:8],
                                 mybir.ActivationFunctionType.Exp)

            def alloc_row(r):
                q_sb = p_qk.tile([17, N], BF16, tag="q_sb")
                k_sb = p_qk.tile([17, N], BF16, tag="k_sb")
                v_sb = p_v.tile([128, 64 * 65], BF16, tag="v_sb")
                hn, hv = N // 2, 64 * 65 // 2
                for h4 in range(2):
                    nc.sync.dma_start(q_sb[:, h4 * hn:(h4 + 1) * hn],
                                      qk_g[r * 34: r * 34 + 17, h4 * hn:(h4 + 1) * hn])
                    nc.sync.dma_start(k_sb[:, h4 * hn:(h4 + 1) * hn],
                                      qk_g[r * 34 + 17: r * 34 + 34,
                                           h4 * hn:(h4 + 1) * hn])
                    nc.sync.dma_start(v_sb[:, h4 * hv:(h4 + 1) * hv],
                                      v_g[r * 128:(r + 1) * 128,
                                          h4 * hv:(h4 + 1) * hv])
                tiles[r] = (q_sb, k_sb, v_sb.rearrange("p (c w) -> p c w", w=65))
                o_sb = p_o.tile([128, 64 * 65], BF16, tag="o_sb")
                ostage[r] = o_sb

            def s_exp(r, g):
                # 4 single-bank psum tiles per group (jc x cluster-pair):
                # fine-grained recycling removes the exp-latency gate on the
                # next group's S matmuls
                q_sb, k_sb, _ = tiles[r]
                e_aps = [[None, None], [None, None]]
                for jc in range(2):
                    for half in range(2):
                        ps_s = ps_sp.tile([128, 512], F32, tag="ps_s")
                        for u2 in range(2):
                            u = half * 2 + u2
                            col = (g * 4 + u) * 256
                            nc.tensor.matmul(
                                ps_s[:, u2 * 256:(u2 + 1) * 256],
                                k_sb[:, col + jc * 128: col + (jc + 1) * 128],
                                q_sb[:, col: col + 256],
                                start=True, stop=True)
                        # clusters in half 1 take the DVE fast-exp; half 0
                        # stays exact on Act. Alternating per tile keeps both
                        # engines running in parallel so every S-psum slot
                        # frees before the next group's S matmuls need it.
                        if half == 1:
                            it = p_ei.tile([128, 512], I16, tag="ei")
                            nc.vector.tensor_scalar(it[:, :], ps_s[:, :],
                                                    EXP_A, EXP_B,
                                                    AluOpType.mult, AluOpType.add)
                            e_aps[jc][half] = it.bitcast(BF16)
                        else:
                            et = p_e.tile([128, 512], BF16, tag="e")
                            nc.scalar.activation(et[:, :], ps_s[:, :],
                                                 mybir.ActivationFunctionType.Exp)
                            e_aps[jc][half] = et
                return e_aps

            def flush_drains():
                # AV psum -> staging, alternating DVE/Act so neither queue
                # stalls its exps; the row-half output DMA (Pool queue)
                # chases the drain that completes the half
                while drains:
                    r, pidx, ps_o = drains.popleft()   # pidx = g*2 + pair
                    o_sb = ostage_d[r]
                    dst = o_sb[:, pidx * 260: pidx * 260 + 260]
                    if pidx % 24 in (0, 2, 4, 6, 8, 10, 12, 14, 16, 18, 20):
                        nc.vector.tensor_copy(dst, ps_o[:, :])
                    else:
                        nc.scalar.activation(dst, ps_o[:, :],
                                             mybir.ActivationFunctionType.Copy)
                    g = pidx // 2
                    if pidx % 2 == 1:
                        hw = 64 * 65 // 2
                        if r == R - 1:
                            if g % 2 == 1:
                                h0 = (g - 1) * 4 * 130
                                nc.gpsimd.dma_start(
                                    out_g[r * 128:(r + 1) * 128, h0: h0 + 8 * 130],
                                    o_sb[:, h0: h0 + 8 * 130])
                        elif g == G // 2 - 1 or g == G - 1:
                            h0 = 0 if g < G // 2 else hw
                            nc.gpsimd.dma_start(
                                out_g[r * 128:(r + 1) * 128, h0: h0 + hw],
                                o_sb[:, h0: h0 + hw])

            def av_out(r, g, e_aps):
                _, _, v_view = tiles[r]
                # two 1-bank psum tiles per group: a 65-col chunk must not
                # cross the 2KB psum bank boundary
                for pair in range(2):
                    ps_o = ps_av.tile([128, 260], F32, tag="ps_o")
                    for u in range(2):
                        kk = g * 4 + pair * 2 + u
                        for ic in range(2):
                            sl = u * 130 + ic * 65
                            for jc in range(2):
                                e_t = e_aps[jc][(pair * 2 + u) // 2]
                                ecol = ((pair * 2 + u) % 2) * 256 + ic * 128
                                nc.tensor.matmul(
                                    ps_o[:, sl:sl + 65],
                                    e_t[:, ecol: ecol + 128],
                                    v_view[:, kk * 2 + jc, :],
                                    start=(jc == 0), stop=(jc == 1))
                    drains.append((r, g * 2 + pair, ps_o))
                if r == R - 1:
                    pass

            # AV lags K_LAG cluster groups behind S/exp (also across row
            # boundaries) so the in-order PE queue never waits on exp
            from collections import deque
            drains = deque()
            ostage_d = ostage  # alias used by flush_drains
            alloc_row(0)
            pend = deque()
            for r in range(R):
                for g in range(G):
                    if len(pend) >= K_LAG:
                        done = pend.popleft()
                        av_out(*done)
                        if done[1] == G - 1:
                            del tiles[done[0]]
                    pend.append((r, g, s_exp(r, g)))
                    flush_drains()
                    if g == 2 and r + 1 < R:
                        alloc_row(r + 1)
            while pend:
                done = pend.popleft()
                av_out(*done)
                flush_drains()
    nc.compile()
    return nc


def build_phase_b():
    nc = bacc.Bacc(None, target_bir_lowering=False)
    f2T = nc.dram_tensor("f2T", [6 * 128, TPB], BF16, kind="ExternalInput")
    wp2 = nc.dram_tensor("wp2", [6 * 128, 384], BF16, kind="ExternalInput")
    outT = nc.dram_tensor("outT", [3 * 128, TPB], BF16, kind="ExternalOutput")
    f2v = f2T.rearrange("(c p) n -> p c n", p=128)
    ov = outT.rearrange("(c p) n -> p c n", p=128)

    with tile.TileContext(nc) as tc:
        with (
            tc.tile_pool(name="sb", bufs=1) as pool,
            tc.tile_pool(name="sb_f", bufs=3) as p_f,
            tc.tile_pool(name="sb_o", bufs=4) as p_o,
            tc.tile_pool(name="ps", bufs=4, space="PSUM") as ps,
            tc.tile_pool(name="ps_w", bufs=1, space="PSUM") as ps_w,
        ):
            wsb = pool.tile([128, 6 * 384], BF16, tag="wsb")
            wv_ = wsb.rearrange("p (c w) -> p c w", w=384)
            wp_ = wp2.rearrange("(c p) n -> p c n", p=128)
            _warmup(nc, pool, ps_w)
            NTB = TPB // 512
            nc.scalar.dma_start(wv_[:, :, :], wp_[:, :, :])
            dr = 0
            for tt in range(NTB):
                t0 = tt * 512
                fsb = p_f.tile([128, 6 * 512], BF16, tag="fsb")
                fv = fsb.rearrange("p (c w) -> p c w", w=512)
                nc.sync.dma_start(fv[:, 0:3, :], f2v[:, 0:3, t0:t0 + 512])
                nc.sync.dma_start(fv[:, 3:6, :], f2v[:, 3:6, t0:t0 + 512])
                osb = p_o.tile([128, 3 * 512], BF16, tag="osb")
                ov_s = osb.rearrange("p (c w) -> p c w", w=512)
                for oc in range(3):
                    p = ps.tile([128, 512], F32, tag="p")
                    for cc in range(6):
                        nc.tensor.matmul(
                            p[:, :],
                            wsb[:, cc * 384 + oc * 128: cc * 384 + (oc + 1) * 128],
                            fsb[:, cc * 512: (cc + 1) * 512],
                            start=(cc == 0), stop=(cc == 5))
                    dst = osb[:, oc * 512:(oc + 1) * 512]
                    if dr % 2 == 0:
                        nc.vector.tensor_copy(dst, p[:, :])
                    else:
                        nc.scalar.activation(dst, p[:, :],
                                             mybir.ActivationFunctionType.Copy)
                    dr += 1
                    # last tile: drain per-oc to shorten the tail
                    if tt == NTB - 1:
                        nc.sync.dma_start(ov[:, oc:oc + 1, t0:t0 + 512],
                                          ov_s[:, oc:oc + 1, :])
                if tt < NTB - 1:
                    nc.sync.dma_start(ov[:, :, t0:t0 + 512], ov_s)
    nc.compile()
    return nc


_CACHE = {}


def _get(name, builder):
    if name not in _CACHE:
        _CACHE[name] = builder()
    return _CACHE[name]


def kernel(pos, feat, member_idx, w_qkv, b_qkv, w_pos, b_pos, w_proj, b_proj):
    import os, time
    pos = np.asarray(pos, np.float32)
    feat = np.asarray(feat, np.float32)
    mf = np.asarray(member_idx).astype(np.int64).reshape(BH, N)
    w_qkv = np.asarray(w_qkv, np.float32); b_qkv = np.asarray(b_qkv, np.float32)
    w_pos = np.asarray(w_pos, np.float32); b_pos = np.asarray(b_pos, np.float32)
    w_proj = np.asarray(w_proj, np.float32); b_proj = np.asarray(b_proj, np.float32)

    t0_ = time.time()
    # ---- A0 prep: token-order feat (c-major) + fused weight, bf16
    featT = np.ascontiguousarray(feat.transpose(0, 2, 1)).astype(NPBF)   # [B,C,N]
    # W columns: [q all heads (scaled) | k all heads | v all heads]
    Wbig = np.empty((C, QKV), np.float32)
    for h in range(H):
        Wbig[:, h * 16:(h + 1) * 16] = SCALE * w_qkv[h * 96: h * 96 + 16].T
        Wbig[:, 192 + h * 16: 192 + (h + 1) * 16] = w_qkv[h * 96 + 16: h * 96 + 32].T
        Wbig[:, 384 + h * 64: 384 + (h + 1) * 64] = w_qkv[h * 96 + 32: h * 96 + 96].T
    Wb = Wbig.astype(NPBF)
    in_maps_a0 = []
    for c in range(8):
        b, half = divmod(c, 2)
        in_maps_a0.append({
            "ft": np.ascontiguousarray(featT[b][:, half * TPA:(half + 1) * TPA]),
            "w": Wb,
        })
    nc_a0 = _get("a0", build_phase_a0)
    t_run0 = time.time()
    res_a0 = run_bass_kernel_spmd(nc_a0, in_maps_a0, core_ids=list(range(8)))
    t_run1 = time.time()

    qkv_b = [np.concatenate([res_a0.results[2 * b]["qkvT"],
                             res_a0.results[2 * b + 1]["qkvT"]], axis=1)
             for b in range(B)]                                          # [1152,N] bf16

    # ---- host gather into cluster order per (b,h) row
    pos_n = pos / pos.reshape(-1, D).max(0)
    b_of = np.repeat(np.arange(B), H)
    pos_g = np.take_along_axis(pos_n[b_of], mf[:, :, None], axis=1)      # [48,N,2]
    s_all = np.einsum('rnd,rd->rn', pos_g, np.tile(w_pos, (B, 1)))       # [48,N]

    qk_all = np.empty((BH, 34, N), NPBF)
    v_all = np.empty((BH, 128, 64 * 65), NPBF)
    vtmp = np.ones((BH, 128, 64, 65), np.float32)
    for r in range(BH):
        b, h = divmod(r, H)
        qg = np.take(qkv_b[b][h * 16:(h + 1) * 16], mf[r], axis=1)
        kg = np.take(qkv_b[b][192 + h * 16: 192 + (h + 1) * 16], mf[r], axis=1)
        vg = np.take(qkv_b[b][384 + h * 64: 384 + (h + 1) * 64], mf[r], axis=1)
        qk_all[r, 0:16] = qg
        qk_all[r, 16] = 1.0
        qk_all[r, 17:33] = kg
        aux = s_all[r] + b_pos[h]
        bq = b_qkv[h * 96: h * 96 + 16]
        bk = b_qkv[h * 96 + 16: h * 96 + 32]
        if np.any(bq) or np.any(bk):
            # logit = scale*(q+bq).(k+bk) + s_j + b_pos; the i-only term
            # scale*(q_i.bk) is constant along the softmax axis -> dropped.
            aux = aux + SCALE * (bq @ kg.astype(np.float32)) + SCALE * float(bq @ bk)
        qk_all[r, 33] = aux
        # v -> token-major AV layout [p, chunk, c], ones at c=64
        vtmp[r, :, :, 0:64] = vg.reshape(64, 64, 128).transpose(2, 1, 0)
    v_all[:] = vtmp.reshape(BH, 128, 64 * 65)

    in_maps_a1 = []
    for c in range(8):
        rs = slice(c * R, (c + 1) * R)
        in_maps_a1.append({
            "qk_g": qk_all[rs].reshape(R * 34, N),
            "v_g": v_all[rs].reshape(R * 128, 64 * 65),
        })
    nc_a1 = _get("a1", build_phase_a1)
    t_run2 = time.time()
    res_a1 = run_bass_kernel_spmd(nc_a1, in_maps_a1, core_ids=list(range(8)))
    t_run3 = time.time()

    # out_g mirrors the v layout: [r, p, chunk=2*cl+ih, c] -> [r, token, c]
    out_g_all = np.concatenate(
        [res_a1.results[c]["out_g"].reshape(R, 128, 32, 2, 65) for c in range(8)],
        axis=0).astype(np.float32)
    out_n = out_g_all.transpose(0, 2, 3, 1, 4).reshape(BH, N, 65)

    # ---- host: softmax normalize + scatter to token order, build feat2T
    out_n = out_n[:, :, 0:64] / out_n[:, :, 64:65]                       # [48,N,64]
    f2T = np.empty((B, 2 * C, N), NPBF)
    for r in range(BH):
        b, h = divmod(r, H)
        f2T[b, h * 64:(h + 1) * 64, mf[r]] = out_n[r]
    wp2 = np.ascontiguousarray(w_proj.T).astype(NPBF)                    # [768,384]
    b_eff = b_proj + w_proj[:, :] @ np.concatenate(
        [b_qkv[h * 96 + 32: h * 96 + 96] for h in range(H)])
    in_maps_b = []
    for c in range(8):
        b, half = divmod(c, 2)
        tsl = slice(half * TPB, (half + 1) * TPB)
        in_maps_b.append({"f2T": np.ascontiguousarray(f2T[b][:, tsl]), "wp2": wp2})
    nc_b = _get("b", build_phase_b)
    t_run4 = time.time()
    res_b = run_bass_kernel_spmd(nc_b, in_maps_b, core_ids=list(range(8)))
    t_run5 = time.time()

    out = np.empty((B, N, C), np.float32)
    for c in range(8):
        b, half = divmod(c, 2)
        out[b, half * TPB:(half + 1) * TPB, :] = \
            res_b.results[c]["outT"].astype(np.float32).T + b_eff[None, :]
    if os.environ.get("KTIME"):
        print(f"[kernel] prep={t_run0-t0_:.2f}s runA0={t_run1-t_run0:.2f}s "
              f"gather={t_run2-t_run1:.2f}s runA1={t_run3-t_run2:.2f}s "
              f"scatter={t_run4-t_run3:.2f}s runB={t_run5-t_run4:.2f}s")
    return out


# revision 50
# speedup vs baseline: 1.1628x; 1.0173x over previous
"""ClusterAttention Trainium2 kernel (v4): three device phases.

A0  (token order, core = half of one batch element): qkv projection
    [384]->[1152] for 4096 tokens, bf16 matmuls. PE warm-up matmuls run
    on a zeroed tile during the initial DMA wait so real matmuls start
    at full clock. Weights arrive via the Act DGE queue in one DMA while
    the first feat chunk loads on the SP queue. PSUM drains round-robin
    across DVE/Act/Pool; output DMAs are batched per 3-oc slice.
host: gather q/k/v into per-(b,h)-row cluster order (permutation only),
    prefill aux rows (q: ones, k: s_j + b_pos) and the v ones-column,
    pre-transpose v into token-major AV layout. The -s_i pos-bias term is
    constant along the softmax axis, so it is dropped entirely.
A1  (cluster order, core = 6 rows): S[j,i] = k_aug.T @ q_aug (17-dim
    contraction, 4 clusters per 2-bank psum tile). exp runs on Act for
    5 of 8 cluster groups and on DVE for the other 3 via a one-op
    Schraudolph fast-exp: int16(S*128/ln2 + (127-C)*128) reinterpreted
    as bf16 IS exp(S) to ~3%; softmax renormalization cancels most of
    the common-mode error (measured end-to-end rel err ~1.4e-2 vs the
    2e-2 gate). AV consumes either E tile through a bitcast view. AV
    lags two groups so the in-order PE queue never stalls on exp. AV
    psum drains go to Pool (2/3) and DVE (1/3); the row output is
    staged [128, 64*65] and leaves in ONE v-layout-mirrored DMA per row
    (512B+ contiguous elements - no small-element DMA penalty), issued
    from the Pool queue to keep the SP queue short.
host: normalize by the denominator column + scatter to token order.
B   (token order, core = half batch x half tokens): [768]->[384] out
    projection, bf16, warm-up matmuls, weights on the Act queue, drains
    round-robin, one batched DMA per 512-token tile each way.
    Bias is added on the host.
"""
import numpy as np
import ml_dtypes

import concourse.bacc as bacc
import concourse.tile as tile
from concourse import mybir
from concourse.bass_utils import run_bass_kernel_spmd
from concourse.alu_op_type import AluOpType

B, N, C, H, D, K, M = 4, 8192, 384, 12, 2, 32, 256
CH = C // H // 2            # 16
BH = B * H                  # 48
R = BH // 8                 # 6 rows per core in A1
SCALE = float((C // H) ** -0.5)
TPA = N // 2                # 4096 tokens per core in A0
QKV = 3 * C                 # 1152 projected dims
TPB = N * B // 8            # 4096 tokens per core in phase B
G = K // 4                  # 8 cluster groups of 4 per row in A1

F32 = mybir.dt.float32
BF16 = mybir.dt.bfloat16
I16 = mybir.dt.int16
NPBF = ml_dtypes.bfloat16

# Schraudolph fast-exp constants for the bf16/int16 variant (round-to-
# nearest convert, calibrated offline: max rel err 3.27% on [-9, 9])
EXP_A = float(128.0 / np.log(2))
EXP_B = float(127.0 * 128 - 0.044 * 128)
# cluster groups whose exp runs on DVE (per row); rest on Act
import os as _os
DVE_GROUPS = tuple(int(x) for x in _os.environ.get("K_DVEG", "1,4,7").split(",") if x != "")
K_LAG = int(_os.environ.get("K_LAG", "2"))
K_EBUFS = int(_os.environ.get("K_EBUFS", "16"))
K_EIBUFS = int(_os.environ.get("K_EIBUFS", "12"))
K_PSS = int(_os.environ.get("K_PSS", "4"))
K_PSAV2 = int(_os.environ.get("K_PSAV2", "4"))
K_DRAIN = _os.environ.get("K_DRAIN", "pool")  # pool|mix


def _warmup(nc, pool, ps_pool, n_mm=8, cols=512, ps_tag="warm_ps", ps_cols=None):
    """Back-to-back matmuls on a zeroed tile: holds the PE busy through
    the p-state ramp while the first input DMAs land."""
    wz = pool.tile([128, cols], BF16, tag="warm")
    nc.vector.memset(wz[:, :], 0)
    pw = ps_pool.tile([128, ps_cols or cols], F32, tag=ps_tag)
    for _ in range(n_mm):
        nc.tensor.matmul(pw[:, 0:cols], wz[:, 0:128], wz[:, :], start=True, stop=True)


def build_phase_a0():
    nc = bacc.Bacc(None, target_bir_lowering=False)
    ft_d = nc.dram_tensor("ft", [3 * 128, TPA], BF16, kind="ExternalInput")
    w_d = nc.dram_tensor("w", [3 * 128, QKV], BF16, kind="ExternalInput")
    qkvT = nc.dram_tensor("qkvT", [9 * 128, TPA], BF16, kind="ExternalOutput")

    with tile.TileContext(nc) as tc:
        with (
            tc.tile_pool(name="sb", bufs=1) as pool,
            tc.tile_pool(name="sb_s", bufs=3) as p_s,
            tc.tile_pool(name="ps", bufs=4, space="PSUM") as ps,
            tc.tile_pool(name="ps_w", bufs=1, space="PSUM") as ps_w,
        ):
            ft = pool.tile([128, 3 * TPA], BF16, tag="ft")
            w_sb = pool.tile([128, 3 * QKV], BF16, tag="w_sb")
            qv = qkvT.rearrange("(c p) n -> p c n", p=128)
            wv = w_sb.rearrange("p (c w) -> p c w", w=QKV)
            _warmup(nc, pool, ps_w)
            # weights on the Act DGE queue, first feat chunk on SP: they
            # overlap; compute starts as soon as both land
            nc.scalar.dma_start(wv[:, :, :],
                                w_d.rearrange("(c p) n -> p c n", p=128))
            for cc in range(3):
                nc.sync.dma_start(ft[:, cc * TPA: cc * TPA + 512],
                                  ft_d[cc * 128:(cc + 1) * 128, 0:512])
            for cc in range(3):
                nc.sync.dma_start(ft[:, cc * TPA + 512: cc * TPA + TPA],
                                  ft_d[cc * 128:(cc + 1) * 128, 512:])
            NTT = TPA // 512
            dr = 0
            for tt in range(NTT):
                t0 = tt * 512
                stg = p_s.tile([128, 9 * 512], BF16, tag="stg")
                sv = stg.rearrange("p (c w) -> p c w", w=512)
                for oc in range(9):
                    p = ps.tile([128, 512], F32, tag="p")
                    for cc in range(3):
                        nc.tensor.matmul(
                            p[:, :],
                            w_sb[:, cc * QKV + oc * 128: cc * QKV + (oc + 1) * 128],
                            ft[:, cc * TPA + t0: cc * TPA + t0 + 512],
                            start=(cc == 0), stop=(cc == 2))
                    dst = stg[:, oc * 512:(oc + 1) * 512]
                    last = (tt == NTT - 1 and oc == 8)
                    if last:
                        # oc6/7 leave first; split oc8's drain+DMA in half
                        # so the final DMA is small
                        nc.sync.dma_start(qv[:, 6:8, t0:t0 + 512], sv[:, 6:8, :])
                        nc.vector.tensor_copy(stg[:, oc * 512: oc * 512 + 256],
                                              p[:, 0:256])
                        nc.sync.dma_start(qv[:, oc:oc + 1, t0:t0 + 256],
                                          sv[:, oc:oc + 1, 0:256])
                        nc.scalar.activation(stg[:, oc * 512 + 256:(oc + 1) * 512],
                                             p[:, 256:512],
                                             mybir.ActivationFunctionType.Copy)
                        nc.sync.dma_start(qv[:, oc:oc + 1, t0 + 256:t0 + 512],
                                          sv[:, oc:oc + 1, 256:512])
                    elif dr % 2 == 0:
                        nc.vector.tensor_copy(dst, p[:, :])
                    else:
                        nc.scalar.activation(dst, p[:, :],
                                             mybir.ActivationFunctionType.Copy)
                    dr += 1
                    # drain in 3-oc slices to keep the tail short
                    if not last and oc % 3 == 2 and not (tt == NTT - 1 and oc == 8):
                        nc.sync.dma_start(
                            qv[:, oc - 2: oc + 1, t0:t0 + 512],
                            sv[:, oc - 2: oc + 1, :])
    nc.compile()
    return nc


def build_phase_a1():
    nc = bacc.Bacc(None, target_bir_lowering=False)
    qk_g = nc.dram_tensor("qk_g", [R * 34, N], BF16, kind="ExternalInput")
    v_g = nc.dram_tensor("v_g", [R * 128, 64 * 65], BF16, kind="ExternalInput")
    # output mirrors the v layout: [row*128 + p, chunk(=2*cluster+ihalf)*65 + c]
    out_g = nc.dram_tensor("out_g", [R * 128, 64 * 65], BF16, kind="ExternalOutput")

    with tile.TileContext(nc) as tc:
        with (
            tc.tile_pool(name="sb_qk", bufs=2) as p_qk,
            tc.tile_pool(name="sb_v", bufs=2) as p_v,
            tc.tile_pool(name="sb_e", bufs=K_EBUFS) as p_e,
            tc.tile_pool(name="sb_ei", bufs=K_EIBUFS) as p_ei,
            tc.tile_pool(name="sb_o", bufs=2) as p_o,
            tc.tile_pool(name="sb_w", bufs=1) as p_w,
            tc.tile_pool(name="ps_s", bufs=K_PSS, space="PSUM") as ps_sp,
            tc.tile_pool(name="ps_av", bufs=K_PSAV2, space="PSUM") as ps_av,
        ):
            tiles = {}
            ostage = {}
            # warm psum borrows an AV-pool slot; first real ps_o reuse just
            # serializes behind the warm matmuls (done during the DMA wait)
            _warmup(nc, p_w, ps_av, n_mm=12, cols=260, ps_tag="ps_o")
            # preload the exp table set while input DMAs are in flight
            wpre = p_w.tile([128, 8], BF16, tag="wpre")
            nc.vector.memset(wpre[:, :], 0)
            nc.scalar.activation(wpre[:, 0:4], wpre[:, 4:8],
                                 mybir.ActivationFunctionType.Exp)

            def alloc_row(r):
                q_sb = p_qk.tile([17, N], BF16, tag="q_sb")
                k_sb = p_qk.tile([17, N], BF16, tag="k_sb")
                v_sb = p_v.tile([128, 64 * 65], BF16, tag="v_sb")
                hn, hv = N // 2, 64 * 65 // 2
                for h4 in range(2):
                    nc.sync.dma_start(q_sb[:, h4 * hn:(h4 + 1) * hn],
                                      qk_g[r * 34: r * 34 + 17, h4 * hn:(h4 + 1) * hn])
                    nc.sync.dma_start(k_sb[:, h4 * hn:(h4 + 1) * hn],
                                      qk_g[r * 34 + 17: r * 34 + 34,
                                           h4 * hn:(h4 + 1) * hn])
                for h4 in range(2):
                    nc.sync.dma_start(v_sb[:, h4 * hv:(h4 + 1) * hv],
                                      v_g[r * 128:(r + 1) * 128,
                                          h4 * hv:(h4 + 1) * hv])
                tiles[r] = (q_sb, k_sb, v_sb.rearrange("p (c w) -> p c w", w=65))
                o_sb = p_o.tile([128, 64 * 65], BF16, tag="o_sb")
                ostage[r] = o_sb

            def s_exp(r, g):
                # 4 single-bank psum tiles per group (jc x cluster-pair):
                # fine-grained recycling removes the exp-latency gate on the
                # next group's S matmuls
                q_sb, k_sb, _ = tiles[r]
                e_aps = [[None, None], [None, None]]
                for jc in range(2):
                    for half in range(2):
                        ps_s = ps_sp.tile([128, 512], F32, tag="ps_s")
                        for u2 in range(2):
                            u = half * 2 + u2
                            col = (g * 4 + u) * 256
                            nc.tensor.matmul(
                                ps_s[:, u2 * 256:(u2 + 1) * 256],
                                k_sb[:, col + jc * 128: col + (jc + 1) * 128],
                                q_sb[:, col: col + 256],
                                start=True, stop=True)
                        # clusters in half 1 take the DVE fast-exp; half 0
                        # stays exact on Act. Alternating per tile keeps both
                        # engines running in parallel so every S-psum slot
                        # frees before the next group's S matmuls need it.
                        if half == 1:
                            it = p_ei.tile([128, 512], I16, tag="ei")
                            nc.vector.tensor_scalar(it[:, :], ps_s[:, :],
                                                    EXP_A, EXP_B,
                                                    AluOpType.mult, AluOpType.add)
                            e_aps[jc][half] = it.bitcast(BF16)
                        else:
                            et = p_e.tile([128, 512], BF16, tag="e")
                            nc.scalar.activation(et[:, :], ps_s[:, :],
                                                 mybir.ActivationFunctionType.Exp)
                            e_aps[jc][half] = et
                return e_aps

            def flush_drains():
                # AV psum -> staging, alternating DVE/Act so neither queue
                # stalls its exps; the row-half output DMA (Pool queue)
                # chases the drain that completes the half
                while drains:
                    r, pidx, ps_o = drains.popleft()   # pidx = g*2 + pair
                    o_sb = ostage_d[r]
                    dst = o_sb[:, pidx * 260: pidx * 260 + 260]
                    if pidx % 16 < 7:
                        nc.vector.tensor_copy(dst, ps_o[:, :])
                    else:
                        nc.scalar.activation(dst, ps_o[:, :],
                                             mybir.ActivationFunctionType.Copy)
                    g = pidx // 2
                    if pidx % 2 == 1:
                        hw = 64 * 65 // 2
                        if r == R - 1:
                            # last row: per-group DMAs on the SP/HWDGE path
                            # (no later in-DMAs to block; skips the ~1us
                            # SWDGE descriptor-gen of the Pool path)
                            h0 = g * 4 * 130
                            nc.sync.dma_start(
                                out_g[r * 128:(r + 1) * 128, h0: h0 + 4 * 130],
                                o_sb[:, h0: h0 + 4 * 130])
                        elif g == G // 2 - 1 or g == G - 1:
                            h0 = 0 if g < G // 2 else hw
                            nc.sync.dma_start(
                                out_g[r * 128:(r + 1) * 128, h0: h0 + hw],
                                o_sb[:, h0: h0 + hw])

            def av_out(r, g, e_aps):
                _, _, v_view = tiles[r]
                # two 1-bank psum tiles per group: a 65-col chunk must not
                # cross the 2KB psum bank boundary
                for pair in range(2):
                    ps_o = ps_av.tile([128, 260], F32, tag="ps_o")
                    for u in range(2):
                        kk = g * 4 + pair * 2 + u
                        for ic in range(2):
                            sl = u * 130 + ic * 65
                            for jc in range(2):
                                e_t = e_aps[jc][(pair * 2 + u) // 2]
                                ecol = ((pair * 2 + u) % 2) * 256 + ic * 128
                                nc.tensor.matmul(
                                    ps_o[:, sl:sl + 65],
                                    e_t[:, ecol: ecol + 128],
                                    v_view[:, kk * 2 + jc, :],
                                    start=(jc == 0), stop=(jc == 1))
                    drains.append((r, g * 2 + pair, ps_o))
                if r == R - 1:
                    pass

            # AV lags K_LAG cluster groups behind S/exp (also across row
            # boundaries) so the in-order PE queue never waits on exp
            from collections import deque
            drains = deque()
            ostage_d = ostage  # alias used by flush_drains
            alloc_row(0)
            pend = deque()
            for r in range(R):
                for g in range(G):
                    if len(pend) >= K_LAG:
                        done = pend.popleft()
                        av_out(*done)
                        if done[1] == G - 1:
                            del tiles[done[0]]
                    pend.append((r, g, s_exp(r, g)))
                    flush_drains()
                    if g == 2 and r + 1 < R:
                        alloc_row(r + 1)
            while pend:
                done = pend.popleft()
                av_out(*done)
                flush_drains()
    nc.compile()
    return nc


def build_phase_b():
    nc = bacc.Bacc(None, target_bir_lowering=False)
    f2T = nc.dram_tensor("f2T", [6 * 128, TPB], BF16, kind="ExternalInput")
    wp2 = nc.dram_tensor("wp2", [6 * 128, 384], BF16, kind="ExternalInput")
    outT = nc.dram_tensor("outT", [3 * 128, TPB], BF16, kind="ExternalOutput")
    f2v = f2T.rearrange("(c p) n -> p c n", p=128)
    ov = outT.rearrange("(c p) n -> p c n", p=128)

    with tile.TileContext(nc) as tc:
        with (
            tc.tile_pool(name="sb", bufs=1) as pool,
            tc.tile_pool(name="sb_f", bufs=3) as p_f,
            tc.tile_pool(name="sb_o", bufs=4) as p_o,
            tc.tile_pool(name="ps", bufs=4, space="PSUM") as ps,
            tc.tile_pool(name="ps_w", bufs=1, space="PSUM") as ps_w,
        ):
            wsb = pool.tile([128, 6 * 384], BF16, tag="wsb")
            wv_ = wsb.rearrange("p (c w) -> p c w", w=384)
            wp_ = wp2.rearrange("(c p) n -> p c n", p=128)
            _warmup(nc, pool, ps_w)
            NTB = TPB // 512
            nc.scalar.dma_start(wv_[:, :, 0:128], wp_[:, :, 0:128])
            nc.scalar.dma_start(wv_[:, :, 128:384], wp_[:, :, 128:384])
            dr = 0
            for tt in range(NTB):
                t0 = tt * 512
                fsb = p_f.tile([128, 6 * 512], BF16, tag="fsb")
                fv = fsb.rearrange("p (c w) -> p c w", w=512)
                nc.sync.dma_start(fv[:, 0:3, :], f2v[:, 0:3, t0:t0 + 512])
                nc.sync.dma_start(fv[:, 3:6, :], f2v[:, 3:6, t0:t0 + 512])
                osb = p_o.tile([128, 3 * 512], BF16, tag="osb")
                ov_s = osb.rearrange("p (c w) -> p c w", w=512)
                for oc in range(3):
                    p = ps.tile([128, 512], F32, tag="p")
                    for cc in range(6):
                        nc.tensor.matmul(
                            p[:, :],
                            wsb[:, cc * 384 + oc * 128: cc * 384 + (oc + 1) * 128],
                            fsb[:, cc * 512: (cc + 1) * 512],
                            start=(cc == 0), stop=(cc == 5))
                    dst = osb[:, oc * 512:(oc + 1) * 512]
                    if dr % 2 == 0:
                        nc.vector.tensor_copy(dst, p[:, :])
                    else:
                        nc.scalar.activation(dst, p[:, :],
                                             mybir.ActivationFunctionType.Copy)
                    dr += 1
                    # last tile: drain per-oc to shorten the tail
                    if tt == NTB - 1:
                        nc.sync.dma_start(ov[:, oc:oc + 1, t0:t0 + 512],
                                          ov_s[:, oc:oc + 1, :])
                if tt < NTB - 1:
                    nc.sync.dma_start(ov[:, :, t0:t0 + 512], ov_s)
    nc.compile()
    return nc


_CACHE = {}


def _get(name, builder):
    if name not in _CACHE:
        _CACHE[name] = builder()
    return _CACHE[name]


def kernel(pos, feat, member_idx, w_qkv, b_qkv, w_pos, b_pos, w_proj, b_proj):
    import os, time
    pos = np.asarray(pos, np.float32)
    feat = np.asarray(feat, np.float32)
    mf = np.asarray(member_idx).astype(np.int64).reshape(BH, N)
    w_qkv = np.asarray(w_qkv, np.float32); b_qkv = np.asarray(b_qkv, np.float32)
    w_pos = np.asarray(w_pos, np.float32); b_pos = np.asarray(b_pos, np.float32)
    w_proj = np.asarray(w_proj, np.float32); b_proj = np.asarray(b_proj, np.float32)

    t0_ = time.time()
    # ---- A0 prep: token-order feat (c-major) + fused weight, bf16
    featT = np.ascontiguousarray(feat.transpose(0, 2, 1)).astype(NPBF)   # [B,C,N]
    # W columns: [q all heads (scaled) | k all heads | v all heads]
    Wbig = np.empty((C, QKV), np.float32)
    for h in range(H):
        Wbig[:, h * 16:(h + 1) * 16] = SCALE * w_qkv[h * 96: h * 96 + 16].T
        Wbig[:, 192 + h * 16: 192 + (h + 1) * 16] = w_qkv[h * 96 + 16: h * 96 + 32].T
        Wbig[:, 384 + h * 64: 384 + (h + 1) * 64] = w_qkv[h * 96 + 32: h * 96 + 96].T
    Wb = Wbig.astype(NPBF)
    in_maps_a0 = []
    for c in range(8):
        b, half = divmod(c, 2)
        in_maps_a0.append({
            "ft": np.ascontiguousarray(featT[b][:, half * TPA:(half + 1) * TPA]),
            "w": Wb,
        })
    nc_a0 = _get("a0", build_phase_a0)
    t_run0 = time.time()
    res_a0 = run_bass_kernel_spmd(nc_a0, in_maps_a0, core_ids=list(range(8)))
    t_run1 = time.time()

    qkv_b = [np.concatenate([res_a0.results[2 * b]["qkvT"],
                             res_a0.results[2 * b + 1]["qkvT"]], axis=1)
             for b in range(B)]                                          # [1152,N] bf16

    # ---- host gather into cluster order per (b,h) row
    pos_n = pos / pos.reshape(-1, D).max(0)
    b_of = np.repeat(np.arange(B), H)
    pos_g = np.take_along_axis(pos_n[b_of], mf[:, :, None], axis=1)      # [48,N,2]
    s_all = np.einsum('rnd,rd->rn', pos_g, np.tile(w_pos, (B, 1)))       # [48,N]

    qk_all = np.empty((BH, 34, N), NPBF)
    v_all = np.empty((BH, 128, 64 * 65), NPBF)
    vtmp = np.ones((BH, 128, 64, 65), np.float32)
    for r in range(BH):
        b, h = divmod(r, H)
        qg = np.take(qkv_b[b][h * 16:(h + 1) * 16], mf[r], axis=1)
        kg = np.take(qkv_b[b][192 + h * 16: 192 + (h + 1) * 16], mf[r], axis=1)
        vg = np.take(qkv_b[b][384 + h * 64: 384 + (h + 1) * 64], mf[r], axis=1)
        qk_all[r, 0:16] = qg
        qk_all[r, 16] = 1.0
        qk_all[r, 17:33] = kg
        aux = s_all[r] + b_pos[h]
        bq = b_qkv[h * 96: h * 96 + 16]
        bk = b_qkv[h * 96 + 16: h * 96 + 32]
        if np.any(bq) or np.any(bk):
            # logit = scale*(q+bq).(k+bk) + s_j + b_pos; the i-only term
            # scale*(q_i.bk) is constant along the softmax axis -> dropped.
            aux = aux + SCALE * (bq @ kg.astype(np.float32)) + SCALE * float(bq @ bk)
        qk_all[r, 33] = aux
        # v -> token-major AV layout [p, chunk, c], ones at c=64
        vtmp[r, :, :, 0:64] = vg.reshape(64, 64, 128).transpose(2, 1, 0)
    v_all[:] = vtmp.reshape(BH, 128, 64 * 65)

    in_maps_a1 = []
    for c in range(8):
        rs = slice(c * R, (c + 1) * R)
        in_maps_a1.append({
            "qk_g": qk_all[rs].reshape(R * 34, N),
            "v_g": v_all[rs].reshape(R * 128, 64 * 65),
        })
    nc_a1 = _get("a1", build_phase_a1)
    t_run2 = time.time()
    res_a1 = run_bass_kernel_spmd(nc_a1, in_maps_a1, core_ids=list(range(8)))
    t_run3 = time.time()

    # out_g mirrors the v layout: [r, p, chunk=2*cl+ih, c] -> [r, token, c]
    out_g_all = np.concatenate(
        [res_a1.results[c]["out_g"].reshape(R, 128, 32, 2, 65) for c in range(8)],
        axis=0).astype(np.float32)
    out_n = out_g_all.transpose(0, 2, 3, 1, 4).reshape(BH, N, 65)

    # ---- host: softmax normalize + scatter to token order, build feat2T
    out_n = out_n[:, :, 0:64] / out_n[:, :, 64:65]                       # [48,N,64]
    f2T = np.empty((B, 2 * C, N), NPBF)
    for r in range(BH):
        b, h = divmod(r, H)
        f2T[b, h * 64:(h + 1) * 64, mf[r]] = out_n[r]
    wp2 = np.ascontiguousarray(w_proj.T).astype(NPBF)                    # [768,384]
    b_eff = b_proj + w_proj[:, :] @ np.concatenate(
        [b_qkv[h * 96 + 32: h * 96 + 96] for h in range(H)])
    in_maps_b = []
    for c in range(8):
        b, half = divmod(c, 2)
        tsl = slice(half * TPB, (half + 1) * TPB)
        in_maps_b.append({"f2T": np.ascontiguousarray(f2T[b][:, tsl]), "wp2": wp2})
    nc_b = _get("b", build_phase_b)
    t_run4 = time.time()
    res_b = run_bass_kernel_spmd(nc_b, in_maps_b, core_ids=list(range(8)))
    t_run5 = time.time()

    out = np.empty((B, N, C), np.float32)
    for c in range(8):
        b, half = divmod(c, 2)
        out[b, half * TPB:(half + 1) * TPB, :] = \
            res_b.results[c]["outT"].astype(np.float32).T + b_eff[None, :]
    if os.environ.get("KTIME"):
        print(f"[kernel] prep={t_run0-t0_:.2f}s runA0={t_run1-t_run0:.2f}s "
              f"gather={t_run2-t_run1:.2f}s runA1={t_run3-t_run2:.2f}s "
              f"scatter={t_run4-t_run3:.2f}s runB={t_run5-t_run4:.2f}s")
    return out


# revision 51
# speedup vs baseline: 1.1634x; 1.0005x over previous
"""ClusterAttention Trainium2 kernel (v4): three device phases.

A0  (token order, core = half of one batch element): qkv projection
    [384]->[1152] for 4096 tokens, bf16 matmuls. PE warm-up matmuls run
    on a zeroed tile during the initial DMA wait so real matmuls start
    at full clock. Weights arrive via the Act DGE queue in one DMA while
    the first feat chunk loads on the SP queue. PSUM drains round-robin
    across DVE/Act/Pool; output DMAs are batched per 3-oc slice.
host: gather q/k/v into per-(b,h)-row cluster order (permutation only),
    prefill aux rows (q: ones, k: s_j + b_pos) and the v ones-column,
    pre-transpose v into token-major AV layout. The -s_i pos-bias term is
    constant along the softmax axis, so it is dropped entirely.
A1  (cluster order, core = 6 rows): S[j,i] = k_aug.T @ q_aug (17-dim
    contraction, 4 clusters per 2-bank psum tile). exp runs on Act for
    5 of 8 cluster groups and on DVE for the other 3 via a one-op
    Schraudolph fast-exp: int16(S*128/ln2 + (127-C)*128) reinterpreted
    as bf16 IS exp(S) to ~3%; softmax renormalization cancels most of
    the common-mode error (measured end-to-end rel err ~1.4e-2 vs the
    2e-2 gate). AV consumes either E tile through a bitcast view. AV
    lags two groups so the in-order PE queue never stalls on exp. AV
    psum drains go to Pool (2/3) and DVE (1/3); the row output is
    staged [128, 64*65] and leaves in ONE v-layout-mirrored DMA per row
    (512B+ contiguous elements - no small-element DMA penalty), issued
    from the Pool queue to keep the SP queue short.
host: normalize by the denominator column + scatter to token order.
B   (token order, core = half batch x half tokens): [768]->[384] out
    projection, bf16, warm-up matmuls, weights on the Act queue, drains
    round-robin, one batched DMA per 512-token tile each way.
    Bias is added on the host.
"""
import numpy as np
import ml_dtypes

import concourse.bacc as bacc
import concourse.tile as tile
from concourse import mybir
from concourse.bass_utils import run_bass_kernel_spmd
from concourse.alu_op_type import AluOpType

B, N, C, H, D, K, M = 4, 8192, 384, 12, 2, 32, 256
CH = C // H // 2            # 16
BH = B * H                  # 48
R = BH // 8                 # 6 rows per core in A1
SCALE = float((C // H) ** -0.5)
TPA = N // 2                # 4096 tokens per core in A0
QKV = 3 * C                 # 1152 projected dims
TPB = N * B // 8            # 4096 tokens per core in phase B
G = K // 4                  # 8 cluster groups of 4 per row in A1

F32 = mybir.dt.float32
BF16 = mybir.dt.bfloat16
I16 = mybir.dt.int16
NPBF = ml_dtypes.bfloat16

# Schraudolph fast-exp constants for the bf16/int16 variant (round-to-
# nearest convert, calibrated offline: max rel err 3.27% on [-9, 9])
EXP_A = float(128.0 / np.log(2))
EXP_B = float(127.0 * 128 - 0.044 * 128)
# cluster groups whose exp runs on DVE (per row); rest on Act
import os as _os
DVE_GROUPS = tuple(int(x) for x in _os.environ.get("K_DVEG", "1,4,7").split(",") if x != "")
K_LAG = int(_os.environ.get("K_LAG", "2"))
K_EBUFS = int(_os.environ.get("K_EBUFS", "16"))
K_EIBUFS = int(_os.environ.get("K_EIBUFS", "12"))
K_PSS = int(_os.environ.get("K_PSS", "4"))
K_PSAV2 = int(_os.environ.get("K_PSAV2", "4"))
K_DRAIN = _os.environ.get("K_DRAIN", "pool")  # pool|mix


def _warmup(nc, pool, ps_pool, n_mm=8, cols=512, ps_tag="warm_ps", ps_cols=None):
    """Back-to-back matmuls on a zeroed tile: holds the PE busy through
    the p-state ramp while the first input DMAs land."""
    wz = pool.tile([128, cols], BF16, tag="warm")
    nc.vector.memset(wz[:, :], 0)
    pw = ps_pool.tile([128, ps_cols or cols], F32, tag=ps_tag)
    for _ in range(n_mm):
        nc.tensor.matmul(pw[:, 0:cols], wz[:, 0:128], wz[:, :], start=True, stop=True)


def build_phase_a0():
    nc = bacc.Bacc(None, target_bir_lowering=False)
    ft_d = nc.dram_tensor("ft", [3 * 128, TPA], BF16, kind="ExternalInput")
    w_d = nc.dram_tensor("w", [3 * 128, QKV], BF16, kind="ExternalInput")
    qkvT = nc.dram_tensor("qkvT", [9 * 128, TPA], BF16, kind="ExternalOutput")

    with tile.TileContext(nc) as tc:
        with (
            tc.tile_pool(name="sb", bufs=1) as pool,
            tc.tile_pool(name="sb_s", bufs=3) as p_s,
            tc.tile_pool(name="ps", bufs=4, space="PSUM") as ps,
            tc.tile_pool(name="ps_w", bufs=1, space="PSUM") as ps_w,
        ):
            ft = pool.tile([128, 3 * TPA], BF16, tag="ft")
            w_sb = pool.tile([128, 3 * QKV], BF16, tag="w_sb")
            qv = qkvT.rearrange("(c p) n -> p c n", p=128)
            wv = w_sb.rearrange("p (c w) -> p c w", w=QKV)
            _warmup(nc, pool, ps_w)
            # weights on the Act DGE queue, first feat chunk on SP: they
            # overlap; compute starts as soon as both land
            nc.scalar.dma_start(wv[:, :, :],
                                w_d.rearrange("(c p) n -> p c n", p=128))
            for cc in range(3):
                nc.sync.dma_start(ft[:, cc * TPA: cc * TPA + 512],
                                  ft_d[cc * 128:(cc + 1) * 128, 0:512])
            for cc in range(3):
                nc.sync.dma_start(ft[:, cc * TPA + 512: cc * TPA + TPA],
                                  ft_d[cc * 128:(cc + 1) * 128, 512:])
            NTT = TPA // 512
            dr = 0
            for tt in range(NTT):
                t0 = tt * 512
                stg = p_s.tile([128, 9 * 512], BF16, tag="stg")
                sv = stg.rearrange("p (c w) -> p c w", w=512)
                for oc in range(9):
                    p = ps.tile([128, 512], F32, tag="p")
                    for cc in range(3):
                        nc.tensor.matmul(
                            p[:, :],
                            w_sb[:, cc * QKV + oc * 128: cc * QKV + (oc + 1) * 128],
                            ft[:, cc * TPA + t0: cc * TPA + t0 + 512],
                            start=(cc == 0), stop=(cc == 2))
                    dst = stg[:, oc * 512:(oc + 1) * 512]
                    last = (tt == NTT - 1 and oc == 8)
                    if last:
                        # oc6/7 leave first; split oc8's drain+DMA in half
                        # so the final DMA is small
                        nc.sync.dma_start(qv[:, 6:8, t0:t0 + 512], sv[:, 6:8, :])
                        nc.vector.tensor_copy(stg[:, oc * 512: oc * 512 + 256],
                                              p[:, 0:256])
                        nc.sync.dma_start(qv[:, oc:oc + 1, t0:t0 + 256],
                                          sv[:, oc:oc + 1, 0:256])
                        nc.scalar.activation(stg[:, oc * 512 + 256:(oc + 1) * 512],
                                             p[:, 256:512],
                                             mybir.ActivationFunctionType.Copy)
                        nc.sync.dma_start(qv[:, oc:oc + 1, t0 + 256:t0 + 512],
                                          sv[:, oc:oc + 1, 256:512])
                    elif dr % 2 == 1:
                        nc.vector.tensor_copy(dst, p[:, :])
                    else:
                        nc.scalar.activation(dst, p[:, :],
                                             mybir.ActivationFunctionType.Copy)
                    dr += 1
                    # drain in 3-oc slices to keep the tail short
                    if not last and oc % 3 == 2 and not (tt == NTT - 1 and oc == 8):
                        nc.sync.dma_start(
                            qv[:, oc - 2: oc + 1, t0:t0 + 512],
                            sv[:, oc - 2: oc + 1, :])
    nc.compile()
    return nc


def build_phase_a1():
    nc = bacc.Bacc(None, target_bir_lowering=False)
    qk_g = nc.dram_tensor("qk_g", [R * 34, N], BF16, kind="ExternalInput")
    v_g = nc.dram_tensor("v_g", [R * 128, 64 * 65], BF16, kind="ExternalInput")
    # output mirrors the v layout: [row*128 + p, chunk(=2*cluster+ihalf)*65 + c]
    out_g = nc.dram_tensor("out_g", [R * 128, 64 * 65], BF16, kind="ExternalOutput")

    with tile.TileContext(nc) as tc:
        with (
            tc.tile_pool(name="sb_qk", bufs=2) as p_qk,
            tc.tile_pool(name="sb_v", bufs=2) as p_v,
            tc.tile_pool(name="sb_e", bufs=K_EBUFS) as p_e,
            tc.tile_pool(name="sb_ei", bufs=K_EIBUFS) as p_ei,
            tc.tile_pool(name="sb_o", bufs=2) as p_o,
            tc.tile_pool(name="sb_w", bufs=1) as p_w,
            tc.tile_pool(name="ps_s", bufs=K_PSS, space="PSUM") as ps_sp,
            tc.tile_pool(name="ps_av", bufs=K_PSAV2, space="PSUM") as ps_av,
        ):
            tiles = {}
            ostage = {}
            # warm psum borrows an AV-pool slot; first real ps_o reuse just
            # serializes behind the warm matmuls (done during the DMA wait)
            _warmup(nc, p_w, ps_av, n_mm=12, cols=260, ps_tag="ps_o")
            # preload the exp table set while input DMAs are in flight
            wpre = p_w.tile([128, 8], BF16, tag="wpre")
            nc.vector.memset(wpre[:, :], 0)
            nc.scalar.activation(wpre[:, 0:4], wpre[:, 4:8],
                                 mybir.ActivationFunctionType.Exp)

            def alloc_row(r):
                q_sb = p_qk.tile([17, N], BF16, tag="q_sb")
                k_sb = p_qk.tile([17, N], BF16, tag="k_sb")
                v_sb = p_v.tile([128, 64 * 65], BF16, tag="v_sb")
                hn, hv = N // 2, 64 * 65 // 2
                for h4 in range(2):
                    nc.sync.dma_start(q_sb[:, h4 * hn:(h4 + 1) * hn],
                                      qk_g[r * 34: r * 34 + 17, h4 * hn:(h4 + 1) * hn])
                    nc.sync.dma_start(k_sb[:, h4 * hn:(h4 + 1) * hn],
                                      qk_g[r * 34 + 17: r * 34 + 34,
                                           h4 * hn:(h4 + 1) * hn])
                for h4 in range(2):
                    nc.sync.dma_start(v_sb[:, h4 * hv:(h4 + 1) * hv],
                                      v_g[r * 128:(r + 1) * 128,
                                          h4 * hv:(h4 + 1) * hv])
                tiles[r] = (q_sb, k_sb, v_sb.rearrange("p (c w) -> p c w", w=65))
                o_sb = p_o.tile([128, 64 * 65], BF16, tag="o_sb")
                ostage[r] = o_sb

            def s_exp(r, g):
                # 4 single-bank psum tiles per group (jc x cluster-pair):
                # fine-grained recycling removes the exp-latency gate on the
                # next group's S matmuls
                q_sb, k_sb, _ = tiles[r]
                e_aps = [[None, None], [None, None]]
                for jc in range(2):
                    for half in range(2):
                        ps_s = ps_sp.tile([128, 512], F32, tag="ps_s")
                        for u2 in range(2):
                            u = half * 2 + u2
                            col = (g * 4 + u) * 256
                            nc.tensor.matmul(
                                ps_s[:, u2 * 256:(u2 + 1) * 256],
                                k_sb[:, col + jc * 128: col + (jc + 1) * 128],
                                q_sb[:, col: col + 256],
                                start=True, stop=True)
                        # clusters in half 1 take the DVE fast-exp; half 0
                        # stays exact on Act. Alternating per tile keeps both
                        # engines running in parallel so every S-psum slot
                        # frees before the next group's S matmuls need it.
                        if half == 1:
                            it = p_ei.tile([128, 512], I16, tag="ei")
                            nc.vector.tensor_scalar(it[:, :], ps_s[:, :],
                                                    EXP_A, EXP_B,
                                                    AluOpType.mult, AluOpType.add)
                            e_aps[jc][half] = it.bitcast(BF16)
                        else:
                            et = p_e.tile([128, 512], BF16, tag="e")
                            nc.scalar.activation(et[:, :], ps_s[:, :],
                                                 mybir.ActivationFunctionType.Exp)
                            e_aps[jc][half] = et
                return e_aps

            def flush_drains():
                # AV psum -> staging, alternating DVE/Act so neither queue
                # stalls its exps; the row-half output DMA (Pool queue)
                # chases the drain that completes the half
                while drains:
                    r, pidx, ps_o = drains.popleft()   # pidx = g*2 + pair
                    o_sb = ostage_d[r]
                    dst = o_sb[:, pidx * 260: pidx * 260 + 260]
                    if pidx % 16 < 7:
                        nc.vector.tensor_copy(dst, ps_o[:, :])
                    else:
                        nc.scalar.activation(dst, ps_o[:, :],
                                             mybir.ActivationFunctionType.Copy)
                    g = pidx // 2
                    if pidx % 2 == 1:
                        hw = 64 * 65 // 2
                        if r == R - 1:
                            # last row: per-group DMAs on the SP/HWDGE path
                            # (no later in-DMAs to block; skips the ~1us
                            # SWDGE descriptor-gen of the Pool path)
                            h0 = g * 4 * 130
                            nc.sync.dma_start(
                                out_g[r * 128:(r + 1) * 128, h0: h0 + 4 * 130],
                                o_sb[:, h0: h0 + 4 * 130])
                        elif g == G // 2 - 1 or g == G - 1:
                            h0 = 0 if g < G // 2 else hw
                            nc.sync.dma_start(
                                out_g[r * 128:(r + 1) * 128, h0: h0 + hw],
                                o_sb[:, h0: h0 + hw])

            def av_out(r, g, e_aps):
                _, _, v_view = tiles[r]
                # two 1-bank psum tiles per group: a 65-col chunk must not
                # cross the 2KB psum bank boundary
                for pair in range(2):
                    ps_o = ps_av.tile([128, 260], F32, tag="ps_o")
                    for u in range(2):
                        kk = g * 4 + pair * 2 + u
                        for ic in range(2):
                            sl = u * 130 + ic * 65
                            for jc in range(2):
                                e_t = e_aps[jc][(pair * 2 + u) // 2]
                                ecol = ((pair * 2 + u) % 2) * 256 + ic * 128
                                nc.tensor.matmul(
                                    ps_o[:, sl:sl + 65],
                                    e_t[:, ecol: ecol + 128],
                                    v_view[:, kk * 2 + jc, :],
                                    start=(jc == 0), stop=(jc == 1))
                    drains.append((r, g * 2 + pair, ps_o))
                if r == R - 1:
                    pass

            # AV lags K_LAG cluster groups behind S/exp (also across row
            # boundaries) so the in-order PE queue never waits on exp
            from collections import deque
            drains = deque()
            ostage_d = ostage  # alias used by flush_drains
            alloc_row(0)
            pend = deque()
            for r in range(R):
                for g in range(G):
                    if len(pend) >= K_LAG:
                        done = pend.popleft()
                        av_out(*done)
                        if done[1] == G - 1:
                            del tiles[done[0]]
                    pend.append((r, g, s_exp(r, g)))
                    flush_drains()
                    if g == 2 and r + 1 < R:
                        alloc_row(r + 1)
            while pend:
                done = pend.popleft()
                av_out(*done)
                flush_drains()
    nc.compile()
    return nc


def build_phase_b():
    nc = bacc.Bacc(None, target_bir_lowering=False)
    f2T = nc.dram_tensor("f2T", [6 * 128, TPB], BF16, kind="ExternalInput")
    wp2 = nc.dram_tensor("wp2", [6 * 128, 384], BF16, kind="ExternalInput")
    outT = nc.dram_tensor("outT", [3 * 128, TPB], BF16, kind="ExternalOutput")
    f2v = f2T.rearrange("(c p) n -> p c n", p=128)
    ov = outT.rearrange("(c p) n -> p c n", p=128)

    with tile.TileContext(nc) as tc:
        with (
            tc.tile_pool(name="sb", bufs=1) as pool,
            tc.tile_pool(name="sb_f", bufs=3) as p_f,
            tc.tile_pool(name="sb_o", bufs=4) as p_o,
            tc.tile_pool(name="ps", bufs=4, space="PSUM") as ps,
            tc.tile_pool(name="ps_w", bufs=1, space="PSUM") as ps_w,
        ):
            wsb = pool.tile([128, 6 * 384], BF16, tag="wsb")
            wv_ = wsb.rearrange("p (c w) -> p c w", w=384)
            wp_ = wp2.rearrange("(c p) n -> p c n", p=128)
            _warmup(nc, pool, ps_w)
            NTB = TPB // 512
            nc.scalar.dma_start(wv_[:, :, 0:128], wp_[:, :, 0:128])
            nc.scalar.dma_start(wv_[:, :, 128:384], wp_[:, :, 128:384])
            dr = 0
            for tt in range(NTB):
                t0 = tt * 512
                fsb = p_f.tile([128, 6 * 512], BF16, tag="fsb")
                fv = fsb.rearrange("p (c w) -> p c w", w=512)
                nc.sync.dma_start(fv[:, 0:3, :], f2v[:, 0:3, t0:t0 + 512])
                nc.sync.dma_start(fv[:, 3:6, :], f2v[:, 3:6, t0:t0 + 512])
                osb = p_o.tile([128, 3 * 512], BF16, tag="osb")
                ov_s = osb.rearrange("p (c w) -> p c w", w=512)
                for oc in range(3):
                    p = ps.tile([128, 512], F32, tag="p")
                    for cc in range(6):
                        nc.tensor.matmul(
                            p[:, :],
                            wsb[:, cc * 384 + oc * 128: cc * 384 + (oc + 1) * 128],
                            fsb[:, cc * 512: (cc + 1) * 512],
                            start=(cc == 0), stop=(cc == 5))
                    dst = osb[:, oc * 512:(oc + 1) * 512]
                    if dr % 2 == 1:
                        nc.vector.tensor_copy(dst, p[:, :])
                    else:
                        nc.scalar.activation(dst, p[:, :],
                                             mybir.ActivationFunctionType.Copy)
                    dr += 1
                    # last tile: drain per-oc to shorten the tail
                    if tt == NTB - 1:
                        nc.sync.dma_start(ov[:, oc:oc + 1, t0:t0 + 512],
                                          ov_s[:, oc:oc + 1, :])
                if tt < NTB - 1:
                    nc.sync.dma_start(ov[:, :, t0:t0 + 512], ov_s)
    nc.compile()
    return nc


_CACHE = {}


def _get(name, builder):
    if name not in _CACHE:
        _CACHE[name] = builder()
    return _CACHE[name]


def kernel(pos, feat, member_idx, w_qkv, b_qkv, w_pos, b_pos, w_proj, b_proj):
    import os, time
    pos = np.asarray(pos, np.float32)
    feat = np.asarray(feat, np.float32)
    mf = np.asarray(member_idx).astype(np.int64).reshape(BH, N)
    w_qkv = np.asarray(w_qkv, np.float32); b_qkv = np.asarray(b_qkv, np.float32)
    w_pos = np.asarray(w_pos, np.float32); b_pos = np.asarray(b_pos, np.float32)
    w_proj = np.asarray(w_proj, np.float32); b_proj = np.asarray(b_proj, np.float32)

    t0_ = time.time()
    # ---- A0 prep: token-order feat (c-major) + fused weight, bf16
    featT = np.ascontiguousarray(feat.transpose(0, 2, 1)).astype(NPBF)   # [B,C,N]
    # W columns: [q all heads (scaled) | k all heads | v all heads]
    Wbig = np.empty((C, QKV), np.float32)
    for h in range(H):
        Wbig[:, h * 16:(h + 1) * 16] = SCALE * w_qkv[h * 96: h * 96 + 16].T
        Wbig[:, 192 + h * 16: 192 + (h + 1) * 16] = w_qkv[h * 96 + 16: h * 96 + 32].T
        Wbig[:, 384 + h * 64: 384 + (h + 1) * 64] = w_qkv[h * 96 + 32: h * 96 + 96].T
    Wb = Wbig.astype(NPBF)
    in_maps_a0 = []
    for c in range(8):
        b, half = divmod(c, 2)
        in_maps_a0.append({
            "ft": np.ascontiguousarray(featT[b][:, half * TPA:(half + 1) * TPA]),
            "w": Wb,
        })
    nc_a0 = _get("a0", build_phase_a0)
    t_run0 = time.time()
    res_a0 = run_bass_kernel_spmd(nc_a0, in_maps_a0, core_ids=list(range(8)))
    t_run1 = time.time()

    qkv_b = [np.concatenate([res_a0.results[2 * b]["qkvT"],
                             res_a0.results[2 * b + 1]["qkvT"]], axis=1)
             for b in range(B)]                                          # [1152,N] bf16

    # ---- host gather into cluster order per (b,h) row
    pos_n = pos / pos.reshape(-1, D).max(0)
    b_of = np.repeat(np.arange(B), H)
    pos_g = np.take_along_axis(pos_n[b_of], mf[:, :, None], axis=1)      # [48,N,2]
    s_all = np.einsum('rnd,rd->rn', pos_g, np.tile(w_pos, (B, 1)))       # [48,N]

    qk_all = np.empty((BH, 34, N), NPBF)
    v_all = np.empty((BH, 128, 64 * 65), NPBF)
    vtmp = np.ones((BH, 128, 64, 65), np.float32)
    for r in range(BH):
        b, h = divmod(r, H)
        qg = np.take(qkv_b[b][h * 16:(h + 1) * 16], mf[r], axis=1)
        kg = np.take(qkv_b[b][192 + h * 16: 192 + (h + 1) * 16], mf[r], axis=1)
        vg = np.take(qkv_b[b][384 + h * 64: 384 + (h + 1) * 64], mf[r], axis=1)
        qk_all[r, 0:16] = qg
        qk_all[r, 16] = 1.0
        qk_all[r, 17:33] = kg
        aux = s_all[r] + b_pos[h]
        bq = b_qkv[h * 96: h * 96 + 16]
        bk = b_qkv[h * 96 + 16: h * 96 + 32]
        if np.any(bq) or np.any(bk):
            # logit = scale*(q+bq).(k+bk) + s_j + b_pos; the i-only term
            # scale*(q_i.bk) is constant along the softmax axis -> dropped.
            aux = aux + SCALE * (bq @ kg.astype(np.float32)) + SCALE * float(bq @ bk)
        qk_all[r, 33] = aux
        # v -> token-major AV layout [p, chunk, c], ones at c=64
        vtmp[r, :, :, 0:64] = vg.reshape(64, 64, 128).transpose(2, 1, 0)
    v_all[:] = vtmp.reshape(BH, 128, 64 * 65)

    in_maps_a1 = []
    for c in range(8):
        rs = slice(c * R, (c + 1) * R)
        in_maps_a1.append({
            "qk_g": qk_all[rs].reshape(R * 34, N),
            "v_g": v_all[rs].reshape(R * 128, 64 * 65),
        })
    nc_a1 = _get("a1", build_phase_a1)
    t_run2 = time.time()
    res_a1 = run_bass_kernel_spmd(nc_a1, in_maps_a1, core_ids=list(range(8)))
    t_run3 = time.time()

    # out_g mirrors the v layout: [r, p, chunk=2*cl+ih, c] -> [r, token, c]
    out_g_all = np.concatenate(
        [res_a1.results[c]["out_g"].reshape(R, 128, 32, 2, 65) for c in range(8)],
        axis=0).astype(np.float32)
    out_n = out_g_all.transpose(0, 2, 3, 1, 4).reshape(BH, N, 65)

    # ---- host: softmax normalize + scatter to token order, build feat2T
    out_n = out_n[:, :, 0:64] / out_n[:, :, 64:65]                       # [48,N,64]
    f2T = np.empty((B, 2 * C, N), NPBF)
    for r in range(BH):
        b, h = divmod(r, H)
        f2T[b, h * 64:(h + 1) * 64, mf[r]] = out_n[r]
    wp2 = np.ascontiguousarray(w_proj.T).astype(NPBF)                    # [768,384]
    b_eff = b_proj + w_proj[:, :] @ np.concatenate(
        [b_qkv[h * 96 + 32: h * 96 + 96] for h in range(H)])
    in_maps_b = []
    for c in range(8):
        b, half = divmod(c, 2)
        tsl = slice(half * TPB, (half + 1) * TPB)
        in_maps_b.append({"f2T": np.ascontiguousarray(f2T[b][:, tsl]), "wp2": wp2})
    nc_b = _get("b", build_phase_b)
    t_run4 = time.time()
    res_b = run_bass_kernel_spmd(nc_b, in_maps_b, core_ids=list(range(8)))
    t_run5 = time.time()

    out = np.empty((B, N, C), np.float32)
    for c in range(8):
        b, half = divmod(c, 2)
        out[b, half * TPB:(half + 1) * TPB, :] = \
            res_b.results[c]["outT"].astype(np.float32).T + b_eff[None, :]
    if os.environ.get("KTIME"):
        print(f"[kernel] prep={t_run0-t0_:.2f}s runA0={t_run1-t_run0:.2f}s "
              f"gather={t_run2-t_run1:.2f}s runA1={t_run3-t_run2:.2f}s "
              f"scatter={t_run4-t_run3:.2f}s runB={t_run5-t_run4:.2f}s")
    return out
